# revision 27
# baseline (speedup 1.0000x reference)
"""MultiHeadEMABlock Trainium2 kernel (8-core SPMD, bass/Tile), t-major rank-r.

Math (reference):
  h = LayerNorm_c(x[b,c,n] over c) * gamma + beta          (per (b,n))
  xe[b,n,h,d] = h[b,n,d] * expansion[h,d]
  y = causal damped EMA along n: y[t] = a_h*sum_{s<=t} q_h^{t-s} xe[s]
  out[b,d,n] = sum_h y[b,n,h,d]*reduction[h,d] + x

Identities:
  - out[c,t] = x[c,t] + sum_h R_h[c]*S_h[t,c], R_h = e_h*r_h*gamma,
    S_h = EMA_{a_h,q_h}(z), z = normalized x (beta handled on host, exact).
  - The actual decay rates are small (q_max ~ 0.57, q^32 < 2e-8), so the
    per-head kernel family {a_h q_h^D, D in [0,256)} has numerical rank ~3:
    a_h q_h^D ~= sum_j U[h,j] G_j(D). Folding per-channel weights
    w_j[c] = sum_h R_h[c] U[h,j] turns the 8-head EMA into r=3 shared
    causal-conv matmuls accumulated in PSUM:
      sum_h R_h (.) S_h ~= sum_j G_j-conv(w_j (.) z)
    Each output chunk needs only its own chunk (intra lhsT, G_j(t-s)) and
    the previous chunk (far lhsT, G_j(t+128-s)): 6 matmuls, no recurrence
    at all since q^128 underflows. The residual rides the same PSUM via an
    identity matmul on x, so the PSUM drain is a single ACT copy.

Layout: host pre-transposes x to t-major [n, c] per core (layout-only prep),
so the device needs NO transposes and LayerNorm stats are per-partition
reductions. Host transposes the t-major output back.

Sharding: 8 cores = 4 batches x 2 sequence halves, 128-row halo (zeros for
the first half; q^128 underflows so this is exact).
"""
import contextlib
import ctypes
import sys
import types

import numpy as np

for _p in ("/root/.axon_site/_ro/trn_rl_repo", "/opt/trn_rl_repo"):
    if _p not in sys.path:
        sys.path.append(_p)

B, C, N, H = 4, 512, 4096, 8
EPS = 1e-5
N_CORES = 8
NHALF = N // 2
L = 128  # chunk length
RNK = 3  # basis rank
NW = NHALF + L  # rows per core incl. halo
NCH = NW // L  # chunks incl. halo chunk
NPAIR = (NCH - 1) // 2  # output chunk pairs


# ---------------------------------------------------------------------------
# axon NTFF shim (lets run_bass_kernel_spmd(trace=True) capture HW profiles)
# ---------------------------------------------------------------------------
def _install_ntff_shim():
    if "antenv.axon_hooks" in sys.modules:
        return
    holder = {"hook": None}

    def _make(so_path):
        try:
            lib = ctypes.CDLL(so_path)
        except OSError:
            return None
        if not hasattr(lib, "axon_start_nrt_profile"):
            return None
        lib.axon_start_nrt_profile.argtypes = [
            ctypes.POINTER(ctypes.c_int64),
            ctypes.c_size_t,
        ]
        lib.axon_start_nrt_profile.restype = ctypes.c_int64
        lib.axon_stop_nrt_profile.argtypes = [ctypes.c_char_p]
        lib.axon_stop_nrt_profile.restype = ctypes.c_int64

        @contextlib.contextmanager
        def _hook(output_dir, device_ids):
            import jax

            jax.devices()
            if device_ids:
                ids = (ctypes.c_int64 * len(device_ids))(*device_ids)
                rc = lib.axon_start_nrt_profile(ids, len(device_ids))
            else:
                rc = lib.axon_start_nrt_profile(None, 0)
            if rc != 0:
                raise RuntimeError(f"axon_start_nrt_profile rc={rc}")
            try:
                yield
            finally:
                n = lib.axon_stop_nrt_profile(str(output_dir).encode())
                print(f"ntff profile: {n} file(s) -> {output_dir}", file=sys.stderr)

        return _hook

    mod = types.ModuleType("antenv.axon_hooks")
    mod.set_axon_ntff_profile_hook = lambda h: holder.__setitem__("hook", h)
    mod.get_axon_ntff_profile_hook = lambda: holder["hook"]
    sys.modules["antenv.axon_hooks"] = mod
    try:
        import antenv

        antenv.axon_hooks = mod
    except ImportError:
        pass
    holder["hook"] = _make("/opt/axon/libaxon_pjrt.so")


def _split_multiwait(nc, max_waits=1):
    """This walrus build rejects >1 sync wait per instruction; split extras
    onto same-engine NoOps inserted just before (per-engine order is the
    execution order, so semantics are preserved)."""
    from concourse import mybir

    k = [0]
    for fn in nc.m.functions:
        for blk in fn.blocks:
            out = []
            for inst in blk.instructions:
                si = getattr(inst, "sync_info", None)
                if si is not None and len(si.on_wait) > max_waits:
                    waits = list(si.on_wait)
                    for w in waits[max_waits:]:
                        k[0] += 1
                        out.append(
                            mybir.InstNoOp(
                                name=f"{inst.name}-mw{k[0]}",
                                sync_info=mybir.SyncInfo(on_wait=[w], on_update=[]),
                                bass_nofuse=True,
                                engine=inst.engine,
                            )
                        )
                    inst.sync_info = mybir.SyncInfo(
                        on_wait=waits[:max_waits], on_update=list(si.on_update)
                    )
                out.append(inst)
            blk.instructions[:] = out


# ---------------------------------------------------------------------------
# program builder
# ---------------------------------------------------------------------------
def build_program():
    import concourse.bass as bass
    import concourse.tile as tile
    from concourse import mybir

    f32 = mybir.dt.float32
    bf16 = mybir.dt.bfloat16
    Op = mybir.AluOpType
    Act = mybir.ActivationFunctionType
    INV_C = 1.0 / C
    SQRT_C = float(np.sqrt(C))

    nc = bass.Bass(
        "TRN2",
        target_bir_lowering=False,
        debug=False,
        enable_asserts=False,
        num_devices=N_CORES,
    )
    xs_d = nc.dram_tensor("xs_t", [128, NCH, C], bf16, kind="ExternalInput").ap()
    # packed consts: [ident | gm0..2 | gf0..2 | wbc0..2(x2 wide)] free-dim packed
    CW = (1 + 2 * RNK) * 128 + RNK * 2 * C
    cm_d = nc.dram_tensor("cmat", [128, CW], bf16, kind="ExternalInput").ap()
    out_d = nc.dram_tensor("out_t", [128, NCH - 1, C], bf16,
                           kind="ExternalOutput").ap()

    with tile.TileContext(nc) as tc:
        with contextlib.ExitStack() as ctx:
            pers = ctx.enter_context(tc.tile_pool(name="pers", bufs=1))
            xb_pool = ctx.enter_context(tc.tile_pool(name="xbp", bufs=4))
            z_pool = ctx.enter_context(tc.tile_pool(name="zp", bufs=6))
            zj_pool = ctx.enter_context(tc.tile_pool(name="zjp", bufs=5))
            st_pool = ctx.enter_context(tc.tile_pool(name="stp", bufs=6))
            out_pool = ctx.enter_context(tc.tile_pool(name="outp", bufs=3))
            ps_pool = ctx.enter_context(tc.tile_pool(name="ps", bufs=1, space="PSUM"))

            # trigger the ACT activation-table load at t~0 with a dummy op so
            # it doesn't delay the first real Square
            epsb = pers.tile([128, 1], f32, tag="eps")
            nc.gpsimd.memset(epsb[:], EPS)
            warm = pers.tile([128, 1], f32, tag="warm")
            nc.scalar.activation(out=warm[:], in_=epsb[:], func=Act.Ln)

            # input slabs + consts interleaved on the sync queue (chunk-major
            # host layout, contiguous per partition): first slab is tiny so
            # the halo chunk's stats start ASAP.
            slab_sizes = [1, 4, 4, 4, 4]  # chunks per slab, sum == NCH
            slab_tiles = [
                xb_pool.tile([128, sw, C], bf16, tag=f"slab{si}", name=f"slab{si}")
                for si, sw in enumerate(slab_sizes)
            ]
            slabs = []
            o = 0
            for sw, sl in zip(slab_sizes, slab_tiles):
                slabs.append((o, sw, sl))
                o += sw

            def xb_view(k):
                for o, sw, sl in slabs:
                    if o <= k < o + sw:
                        return sl[:, k - o, :]
                raise IndexError(k)

            cmat = pers.tile([128, CW], bf16, tag="cmat")
            ident = cmat[:, 0:128]
            gm = [cmat[:, (1 + j) * 128 : (2 + j) * 128] for j in range(RNK)]
            gf = [cmat[:, (1 + RNK + j) * 128 : (2 + RNK + j) * 128]
                  for j in range(RNK)]
            wb0 = (1 + 2 * RNK) * 128
            wbc = [cmat[:, wb0 + j * 2 * C : wb0 + (j + 1) * 2 * C]
                   for j in range(RNK)]

            def slab_dma(si):
                o, sw, sl = slabs[si]
                nc.sync.dma_start(out=sl[:], in_=xs_d[:, o : o + sw, :])

            slab_dma(0)
            nc.sync.dma_start(out=cmat[:], in_=cm_d)
            slab_dma(1)
            slab_dma(2)
            slab_dma(3)
            slab_dma(4)

            def stats(xb_ap, sums, idx):
                ssum, sumsq, mean, m2c, vd, rstd = sums
                sq = st_pool.tile([128, C], bf16, tag="sq")
                nc.scalar.activation(out=sq[:], in_=xb_ap, func=Act.Square,
                                     accum_out=sumsq[:, idx : idx + 1])
                nc.vector.tensor_reduce(out=ssum[:, idx : idx + 1], in_=xb_ap,
                                        axis=mybir.AxisListType.X, op=Op.add)

            def stats_finish(sums, width):
                ssum, sumsq, mean, m2c, vd, rstd = sums
                nc.vector.tensor_scalar(out=mean[:, :width], in0=ssum[:, :width],
                                        scalar1=INV_C, scalar2=None, op0=Op.mult)
                nc.scalar.activation(out=m2c[:, :width], in_=mean[:, :width],
                                     func=Act.Square, scale=SQRT_C)
                nc.vector.tensor_tensor(out=vd[:, :width], in0=sumsq[:, :width],
                                        in1=m2c[:, :width], op=Op.subtract)
                lnv = st_pool.tile([128, 2], f32, tag="lnv")
                nc.scalar.activation(out=lnv[:, :width], in_=vd[:, :width],
                                     func=Act.Ln, scale=INV_C, bias=epsb[:])
                nc.scalar.activation(out=rstd[:, :width], in_=lnv[:, :width],
                                     func=Act.Exp, scale=-0.5)

            def new_sums():
                return tuple(
                    st_pool.tile([128, 2], f32, tag=nm, name=nm)
                    for nm in ("ssum", "sumsq", "mean", "m2c", "vd", "rstd")
                )

            def norm_pair(xb_aps, sums, width):
                """normalize chunks into one flat [128, width*C] tile, then
                produce RNK pair-wide scaled copies (2D ops keep the DVE 2x
                fast path; one instruction covers both chunks)."""
                ssum, sumsq, mean, m2c, vd, rstd = sums
                zt = z_pool.tile([128, 2 * C], bf16, tag="zt")
                for idx, xb_ap in enumerate(xb_aps):
                    nc.vector.tensor_scalar(out=zt[:, idx * C : (idx + 1) * C],
                                            in0=xb_ap,
                                            scalar1=mean[:, idx : idx + 1],
                                            scalar2=rstd[:, idx : idx + 1],
                                            op0=Op.subtract, op1=Op.mult)
                zj = []
                for j in range(RNK):
                    t = zj_pool.tile([128, 2 * C], bf16, tag=f"zj{j}",
                                     name=f"zj{j}")
                    nc.vector.tensor_tensor(out=t[:, : width * C],
                                            in0=zt[:, : width * C],
                                            in1=wbc[j][:, : width * C]
                                            if hasattr(wbc[j], '__getitem__')
                                            else wbc[j], op=Op.mult)
                    zj.append(t)
                return zj

            # ---- halo chunk (k=0): stats + norm + scaled copies only ----
            sums0 = new_sums()
            stats(xb_view(0), sums0, 0)
            stats_finish(sums0, 1)

            # ---- software-pipelined pair stages ----
            # stats run STATS_AHEAD pairs ahead of the norm/matmul stage so the
            # rstd chain is never on the per-chunk critical path; drains are a
            # pair late so the in-order ACT queue never stalls on a hot PSUM.
            STATS_AHEAD = 3
            pair_sums = {}

            def emit_stats(p):
                if p >= NPAIR:
                    return
                sums = new_sums()
                for i in range(2):
                    stats(xb_view(2 * p + 1 + i), sums, i)
                stats_finish(sums, 2)
                pair_sums[p] = sums

            pending = []  # (ema_pair_psum, ot_tile, pair_idx)

            def flush_one():
                ema_, ot_, p_ = pending.pop(0)
                nc.scalar.activation(out=ot_[:], in_=ema_[:], func=Act.Copy)
                nc.scalar.dma_start(
                    out=out_d[:, 2 * p_ : 2 * p_ + 2, :], in_=ot_[:]
                )

            for p in range(STATS_AHEAD):
                emit_stats(p)
            zjh = norm_pair([xb_view(0)], sums0, 1)
            zj_prev = [zjh[j][:, 0:C] for j in range(RNK)]

            for p in range(NPAIR):
                emit_stats(p + STATS_AHEAD)
                sums = pair_sums.pop(p)
                ot = out_pool.tile([128, 2, C], bf16, tag="ot", bufs=3)
                ema = ps_pool.tile([128, 2, C], f32, tag="ema", bufs=4)
                zjp = norm_pair([xb_view(2 * p + 1), xb_view(2 * p + 2)],
                                sums, 2)
                for i in range(2):
                    k = 2 * p + 1 + i
                    zj = [zjp[j][:, i * C : (i + 1) * C] for j in range(RNK)]
                    nc.tensor.matmul(out=ema[:, i, :], lhsT=ident, rhs=xb_view(k),
                                     start=True, stop=False)
                    for j in range(RNK):
                        nc.tensor.matmul(out=ema[:, i, :], lhsT=gf[j],
                                         rhs=zj_prev[j], start=False, stop=False)
                    for j in range(RNK):
                        nc.tensor.matmul(out=ema[:, i, :], lhsT=gm[j], rhs=zj[j],
                                         start=False, stop=(j == RNK - 1))
                    zj_prev = zj
                pending.append((ema, ot, p))
                if len(pending) > 1:
                    flush_one()
            while pending:
                flush_one()
    return nc


def _host_params(ln_gamma, ln_beta, expansion, reduction, alphas, dampen_factors):
    import ml_dtypes

    bf = ml_dtypes.bfloat16
    a = 1.0 / (1.0 + np.exp(-alphas.astype(np.float64)))
    q = (1.0 - a) / (1.0 + np.exp(-dampen_factors.astype(np.float64)))
    R = (
        expansion.astype(np.float64)
        * reduction.astype(np.float64)
        * ln_gamma.astype(np.float64)[None, :]
    )  # [H, C]
    t2 = np.arange(2 * L)
    M2 = a[:, None] * q[:, None] ** t2[None, :]  # [H, 2L]
    bw = np.linalg.norm(R, axis=1)
    bw = np.where(bw > 0, bw, 1.0)
    u, s, vt = np.linalg.svd(M2 * bw[:, None], full_matrices=False)
    G = vt[:RNK] * s[:RNK, None]  # [r, 2L]
    U = u[:, :RNK] / bw[:, None]  # [H, r], M2 ~= U @ G
    w = R.T @ U  # [C, r]
    for j in range(RNK):  # balance scales for bf16
        sc = np.sqrt(np.abs(G[j]).max() / max(np.abs(w[:, j]).max(), 1e-30))
        G[j] /= sc
        w[:, j] *= sc

    blocks = [np.eye(128)]
    gm_blocks, gf_blocks = [], []
    for j in range(RNK):
        Tn = np.zeros((L, L))
        Tf = np.zeros((L, L))
        for s_ in range(L):
            Tn[s_, s_:] = G[j, : L - s_]       # lhsT[s, t] = G_j(t - s)
            Tf[s_, :] = G[j, L - s_ : 2 * L - s_]  # lhsT[s, t] = G_j(t + L - s)
        gm_blocks.append(Tn)
        gf_blocks.append(Tf)
    blocks += gm_blocks + gf_blocks
    for j in range(RNK):
        blocks.append(np.broadcast_to(np.tile(w[:, j], 2)[None, :], (128, 2 * C)))
    cmat = np.concatenate(blocks, axis=1).astype(bf)  # [128, CW]
    consts = dict(cmat=cmat)
    return a, q, consts


def _beta_term(ln_beta, expansion, reduction, a, q):
    if not np.any(ln_beta):
        return None
    n_idx = np.arange(N, dtype=np.float64)
    Cn = a[:, None] * (1.0 - q[:, None] ** (n_idx[None, :] + 1.0)) / (1.0 - q[:, None])
    w = (
        expansion.astype(np.float64)
        * reduction.astype(np.float64)
        * ln_beta.astype(np.float64)[None, :]
    )
    return np.einsum("hc,hn->cn", w, Cn).astype(np.float32)


def _make_in_maps(x, consts):
    import ml_dtypes

    bf = ml_dtypes.bfloat16
    xt = np.ascontiguousarray(np.swapaxes(x, 1, 2)).astype(bf)  # [B, N, C]
    in_maps = []
    for core in range(N_CORES):
        b, half = divmod(core, 2)
        xs = np.zeros((NW, C), bf)
        s = half * NHALF - L
        if s < 0:
            xs[L:] = xt[b, :NHALF]
        else:
            xs[:] = xt[b, s : s + NW]
        # chunk-major [q, k, c] so device DMA slabs are contiguous/partition
        xs_km = np.ascontiguousarray(
            xs.reshape(NCH, 128, C).transpose(1, 0, 2)
        )
        in_maps.append(dict(consts, xs_t=xs_km))
    return in_maps


def kernel(x, ln_gamma, ln_beta, expansion, reduction, alphas, dampen_factors,
           trace=False):
    _install_ntff_shim()
    from concourse.bass_utils import run_bass_kernel_spmd
    from concourse.bass_interp import get_hw_module

    x = np.asarray(x, np.float32)
    a, q, consts = _host_params(
        np.asarray(ln_gamma), np.asarray(ln_beta), np.asarray(expansion),
        np.asarray(reduction), np.asarray(alphas), np.asarray(dampen_factors),
    )
    nc = build_program()
    _split_multiwait(nc)
    nc.m = get_hw_module(nc.m)

    in_maps = _make_in_maps(x, consts)
    res = run_bass_kernel_spmd(
        nc, in_maps, core_ids=list(range(N_CORES)), trace=trace
    )

    out = np.empty((B, C, N), np.float32)
    for core in range(N_CORES):
        b, half = divmod(core, 2)
        # [q, k, c] chunk-major -> [n, c] -> transpose to [c, n]
        ot = (res.results[core]["out_t"].astype(np.float32)
              .transpose(1, 0, 2).reshape(NHALF, C))
        out[b, :, half * NHALF : (half + 1) * NHALF] = ot.T
    bt = _beta_term(
        np.asarray(ln_beta), np.asarray(expansion), np.asarray(reduction), a, q
    )
    if bt is not None:
        out += bt[None]
    if trace:
        kernel.last_results = res
    return out


# revision 28
# speedup vs baseline: 1.0218x; 1.0218x over previous
"""MultiHeadEMABlock Trainium2 kernel (8-core SPMD, bass/Tile), t-major rank-r.

Math (reference):
  h = LayerNorm_c(x[b,c,n] over c) * gamma + beta          (per (b,n))
  xe[b,n,h,d] = h[b,n,d] * expansion[h,d]
  y = causal damped EMA along n: y[t] = a_h*sum_{s<=t} q_h^{t-s} xe[s]
  out[b,d,n] = sum_h y[b,n,h,d]*reduction[h,d] + x

Identities:
  - out[c,t] = x[c,t] + sum_h R_h[c]*S_h[t,c], R_h = e_h*r_h*gamma,
    S_h = EMA_{a_h,q_h}(z), z = normalized x (beta handled on host, exact).
  - The actual decay rates are small (q_max ~ 0.57, q^32 < 2e-8), so the
    per-head kernel family {a_h q_h^D, D in [0,256)} has numerical rank ~3:
    a_h q_h^D ~= sum_j U[h,j] G_j(D). Folding per-channel weights
    w_j[c] = sum_h R_h[c] U[h,j] turns the 8-head EMA into r=3 shared
    causal-conv matmuls accumulated in PSUM:
      sum_h R_h (.) S_h ~= sum_j G_j-conv(w_j (.) z)
    Each output chunk needs only its own chunk (intra lhsT, G_j(t-s)) and
    the previous chunk (far lhsT, G_j(t+128-s)): 6 matmuls, no recurrence
    at all since q^128 underflows. The residual rides the same PSUM via an
    identity matmul on x, so the PSUM drain is a single ACT copy.

Layout: host pre-transposes x to t-major [n, c] per core (layout-only prep),
so the device needs NO transposes and LayerNorm stats are per-partition
reductions. Host transposes the t-major output back.

Sharding: 8 cores = 4 batches x 2 sequence halves, 128-row halo (zeros for
the first half; q^128 underflows so this is exact).
"""
import contextlib
import ctypes
import sys
import types

import numpy as np

for _p in ("/root/.axon_site/_ro/trn_rl_repo", "/opt/trn_rl_repo"):
    if _p not in sys.path:
        sys.path.append(_p)

B, C, N, H = 4, 512, 4096, 8
EPS = 1e-5
N_CORES = 8
NHALF = N // 2
L = 128  # chunk length
RNK = 3  # basis rank
NW = NHALF + L  # rows per core incl. halo
NCH = NW // L  # chunks incl. halo chunk
NPAIR = (NCH - 1) // 2  # output chunk pairs


# ---------------------------------------------------------------------------
# axon NTFF shim (lets run_bass_kernel_spmd(trace=True) capture HW profiles)
# ---------------------------------------------------------------------------
def _install_ntff_shim():
    if "antenv.axon_hooks" in sys.modules:
        return
    holder = {"hook": None}

    def _make(so_path):
        try:
            lib = ctypes.CDLL(so_path)
        except OSError:
            return None
        if not hasattr(lib, "axon_start_nrt_profile"):
            return None
        lib.axon_start_nrt_profile.argtypes = [
            ctypes.POINTER(ctypes.c_int64),
            ctypes.c_size_t,
        ]
        lib.axon_start_nrt_profile.restype = ctypes.c_int64
        lib.axon_stop_nrt_profile.argtypes = [ctypes.c_char_p]
        lib.axon_stop_nrt_profile.restype = ctypes.c_int64

        @contextlib.contextmanager
        def _hook(output_dir, device_ids):
            import jax

            jax.devices()
            if device_ids:
                ids = (ctypes.c_int64 * len(device_ids))(*device_ids)
                rc = lib.axon_start_nrt_profile(ids, len(device_ids))
            else:
                rc = lib.axon_start_nrt_profile(None, 0)
            if rc != 0:
                raise RuntimeError(f"axon_start_nrt_profile rc={rc}")
            try:
                yield
            finally:
                n = lib.axon_stop_nrt_profile(str(output_dir).encode())
                print(f"ntff profile: {n} file(s) -> {output_dir}", file=sys.stderr)

        return _hook

    mod = types.ModuleType("antenv.axon_hooks")
    mod.set_axon_ntff_profile_hook = lambda h: holder.__setitem__("hook", h)
    mod.get_axon_ntff_profile_hook = lambda: holder["hook"]
    sys.modules["antenv.axon_hooks"] = mod
    try:
        import antenv

        antenv.axon_hooks = mod
    except ImportError:
        pass
    holder["hook"] = _make("/opt/axon/libaxon_pjrt.so")


def _split_multiwait(nc, max_waits=1):
    """This walrus build rejects >1 sync wait per instruction; split extras
    onto same-engine NoOps inserted just before (per-engine order is the
    execution order, so semantics are preserved)."""
    from concourse import mybir

    k = [0]
    for fn in nc.m.functions:
        for blk in fn.blocks:
            out = []
            for inst in blk.instructions:
                si = getattr(inst, "sync_info", None)
                if si is not None and len(si.on_wait) > max_waits:
                    waits = list(si.on_wait)
                    for w in waits[max_waits:]:
                        k[0] += 1
                        out.append(
                            mybir.InstNoOp(
                                name=f"{inst.name}-mw{k[0]}",
                                sync_info=mybir.SyncInfo(on_wait=[w], on_update=[]),
                                bass_nofuse=True,
                                engine=inst.engine,
                            )
                        )
                    inst.sync_info = mybir.SyncInfo(
                        on_wait=waits[:max_waits], on_update=list(si.on_update)
                    )
                out.append(inst)
            blk.instructions[:] = out


# ---------------------------------------------------------------------------
# program builder
# ---------------------------------------------------------------------------
def build_program():
    import concourse.bass as bass
    import concourse.tile as tile
    from concourse import mybir

    f32 = mybir.dt.float32
    bf16 = mybir.dt.bfloat16
    Op = mybir.AluOpType
    Act = mybir.ActivationFunctionType
    INV_C = 1.0 / C
    SQRT_C = float(np.sqrt(C))

    nc = bass.Bass(
        "TRN2",
        target_bir_lowering=False,
        debug=False,
        enable_asserts=False,
        num_devices=N_CORES,
    )
    xs_d = nc.dram_tensor("xs_t", [128, NCH, C], bf16, kind="ExternalInput").ap()
    # packed consts: [ident | gm0..2 | gf0..2 | wbc0..2(x2 wide)] free-dim packed
    CW = (1 + 2 * RNK) * 128 + RNK * 2 * C
    cm_d = nc.dram_tensor("cmat", [128, CW], bf16, kind="ExternalInput").ap()
    out_d = nc.dram_tensor("out_t", [128, NCH - 1, C], bf16,
                           kind="ExternalOutput").ap()

    with tile.TileContext(nc) as tc:
        with contextlib.ExitStack() as ctx:
            pers = ctx.enter_context(tc.tile_pool(name="pers", bufs=1))
            xb_pool = ctx.enter_context(tc.tile_pool(name="xbp", bufs=4))
            z_pool = ctx.enter_context(tc.tile_pool(name="zp", bufs=6))
            zj_pool = ctx.enter_context(tc.tile_pool(name="zjp", bufs=5))
            st_pool = ctx.enter_context(tc.tile_pool(name="stp", bufs=6))
            out_pool = ctx.enter_context(tc.tile_pool(name="outp", bufs=3))
            ps_pool = ctx.enter_context(tc.tile_pool(name="ps", bufs=1, space="PSUM"))

            # trigger the ACT activation-table load at t~0 with a dummy op so
            # it doesn't delay the first real Square
            epsb = pers.tile([128, 1], f32, tag="eps")
            nc.gpsimd.memset(epsb[:], EPS)
            warm = pers.tile([128, 1], f32, tag="warm")
            nc.scalar.activation(out=warm[:], in_=epsb[:], func=Act.Ln)

            # input slabs + consts interleaved on the sync queue (chunk-major
            # host layout, contiguous per partition): first slab is tiny so
            # the halo chunk's stats start ASAP.
            slab_sizes = [1, 4, 4, 4, 4]  # chunks per slab, sum == NCH
            slab_tiles = [
                xb_pool.tile([128, sw, C], bf16, tag=f"slab{si}", name=f"slab{si}")
                for si, sw in enumerate(slab_sizes)
            ]
            slabs = []
            o = 0
            for sw, sl in zip(slab_sizes, slab_tiles):
                slabs.append((o, sw, sl))
                o += sw

            def xb_view(k):
                for o, sw, sl in slabs:
                    if o <= k < o + sw:
                        return sl[:, k - o, :]
                raise IndexError(k)

            cmat = pers.tile([128, CW], bf16, tag="cmat")
            ident = cmat[:, 0:128]
            gm = [cmat[:, (1 + j) * 128 : (2 + j) * 128] for j in range(RNK)]
            gf = [cmat[:, (1 + RNK + j) * 128 : (2 + RNK + j) * 128]
                  for j in range(RNK)]
            wb0 = (1 + 2 * RNK) * 128
            wbc = [cmat[:, wb0 + j * 2 * C : wb0 + (j + 1) * 2 * C]
                   for j in range(RNK)]

            def slab_dma(si):
                o, sw, sl = slabs[si]
                nc.sync.dma_start(out=sl[:], in_=xs_d[:, o : o + sw, :])

            slab_dma(0)
            nc.sync.dma_start(out=cmat[:], in_=cm_d)
            slab_dma(1)
            slab_dma(2)
            slab_dma(3)
            slab_dma(4)

            def stats(xb_ap, sums, idx):
                ssum, sumsq, mean, m2c, vd, rstd = sums
                sq = st_pool.tile([128, C], bf16, tag="sq")
                nc.scalar.activation(out=sq[:], in_=xb_ap, func=Act.Square,
                                     accum_out=sumsq[:, idx : idx + 1])
                nc.vector.tensor_reduce(out=ssum[:, idx : idx + 1], in_=xb_ap,
                                        axis=mybir.AxisListType.X, op=Op.add)

            def stats_finish(sums, width):
                ssum, sumsq, mean, m2c, vd, rstd = sums
                nc.vector.tensor_scalar(out=mean[:, :width], in0=ssum[:, :width],
                                        scalar1=INV_C, scalar2=None, op0=Op.mult)
                nc.scalar.activation(out=m2c[:, :width], in_=mean[:, :width],
                                     func=Act.Square, scale=SQRT_C)
                nc.vector.tensor_tensor(out=vd[:, :width], in0=sumsq[:, :width],
                                        in1=m2c[:, :width], op=Op.subtract)
                lnv = st_pool.tile([128, 2], f32, tag="lnv")
                nc.scalar.activation(out=lnv[:, :width], in_=vd[:, :width],
                                     func=Act.Ln, scale=INV_C, bias=epsb[:])
                nc.scalar.activation(out=rstd[:, :width], in_=lnv[:, :width],
                                     func=Act.Exp, scale=-0.5)

            def new_sums():
                return tuple(
                    st_pool.tile([128, 2], f32, tag=nm, name=nm)
                    for nm in ("ssum", "sumsq", "mean", "m2c", "vd", "rstd")
                )

            def norm_pair(xb_aps, sums, width):
                """normalize chunks into one flat [128, width*C] tile, then
                produce RNK pair-wide scaled copies (2D ops keep the DVE 2x
                fast path; one instruction covers both chunks)."""
                ssum, sumsq, mean, m2c, vd, rstd = sums
                zt = z_pool.tile([128, 2 * C], bf16, tag="zt")
                for idx, xb_ap in enumerate(xb_aps):
                    nc.vector.tensor_scalar(out=zt[:, idx * C : (idx + 1) * C],
                                            in0=xb_ap,
                                            scalar1=mean[:, idx : idx + 1],
                                            scalar2=rstd[:, idx : idx + 1],
                                            op0=Op.subtract, op1=Op.mult)
                zj = []
                for j in range(RNK):
                    t = zj_pool.tile([128, 2 * C], bf16, tag=f"zj{j}",
                                     name=f"zj{j}")
                    nc.vector.tensor_tensor(out=t[:, : width * C],
                                            in0=zt[:, : width * C],
                                            in1=wbc[j][:, : width * C]
                                            if hasattr(wbc[j], '__getitem__')
                                            else wbc[j], op=Op.mult)
                    zj.append(t)
                return zj

            # ---- halo chunk (k=0): stats + norm + scaled copies only ----
            sums0 = new_sums()
            stats(xb_view(0), sums0, 0)
            stats_finish(sums0, 1)

            # ---- software-pipelined pair stages ----
            # stats run STATS_AHEAD pairs ahead of the norm/matmul stage so the
            # rstd chain is never on the per-chunk critical path; drains are a
            # pair late so the in-order ACT queue never stalls on a hot PSUM.
            STATS_AHEAD = 3
            pair_sums = {}

            def emit_stats(p):
                if p >= NPAIR:
                    return
                sums = new_sums()
                for i in range(2):
                    stats(xb_view(2 * p + 1 + i), sums, i)
                stats_finish(sums, 2)
                pair_sums[p] = sums

            pending = []  # (ema_pair_psum, ot_tile, pair_idx)

            def flush_one():
                ema_, ot_, p_ = pending.pop(0)
                nc.scalar.activation(out=ot_[:], in_=ema_[:], func=Act.Copy)
                nc.scalar.dma_start(
                    out=out_d[:, 2 * p_ : 2 * p_ + 2, :], in_=ot_[:]
                )

            for p in range(STATS_AHEAD):
                emit_stats(p)
            zjh = norm_pair([xb_view(0)], sums0, 1)
            zj_prev = [zjh[j][:, 0:C] for j in range(RNK)]

            for p in range(NPAIR):
                sums = pair_sums.pop(p)
                ot = out_pool.tile([128, 2, C], bf16, tag="ot", bufs=3)
                ema = ps_pool.tile([128, 2, C], f32, tag="ema", bufs=4)
                zjp = norm_pair([xb_view(2 * p + 1), xb_view(2 * p + 2)],
                                sums, 2)
                for i in range(2):
                    k = 2 * p + 1 + i
                    zj = [zjp[j][:, i * C : (i + 1) * C] for j in range(RNK)]
                    nc.tensor.matmul(out=ema[:, i, :], lhsT=ident, rhs=xb_view(k),
                                     start=True, stop=False)
                    for j in range(RNK):
                        nc.tensor.matmul(out=ema[:, i, :], lhsT=gf[j],
                                         rhs=zj_prev[j], start=False, stop=False)
                    for j in range(RNK):
                        nc.tensor.matmul(out=ema[:, i, :], lhsT=gm[j], rhs=zj[j],
                                         start=False, stop=(j == RNK - 1))
                    zj_prev = zj
                pending.append((ema, ot, p))
                emit_stats(p + STATS_AHEAD)
                if len(pending) > 1:
                    flush_one()
            while pending:
                flush_one()
    return nc


def _host_params(ln_gamma, ln_beta, expansion, reduction, alphas, dampen_factors):
    import ml_dtypes

    bf = ml_dtypes.bfloat16
    a = 1.0 / (1.0 + np.exp(-alphas.astype(np.float64)))
    q = (1.0 - a) / (1.0 + np.exp(-dampen_factors.astype(np.float64)))
    R = (
        expansion.astype(np.float64)
        * reduction.astype(np.float64)
        * ln_gamma.astype(np.float64)[None, :]
    )  # [H, C]
    t2 = np.arange(2 * L)
    M2 = a[:, None] * q[:, None] ** t2[None, :]  # [H, 2L]
    bw = np.linalg.norm(R, axis=1)
    bw = np.where(bw > 0, bw, 1.0)
    u, s, vt = np.linalg.svd(M2 * bw[:, None], full_matrices=False)
    G = vt[:RNK] * s[:RNK, None]  # [r, 2L]
    U = u[:, :RNK] / bw[:, None]  # [H, r], M2 ~= U @ G
    w = R.T @ U  # [C, r]
    for j in range(RNK):  # balance scales for bf16
        sc = np.sqrt(np.abs(G[j]).max() / max(np.abs(w[:, j]).max(), 1e-30))
        G[j] /= sc
        w[:, j] *= sc

    blocks = [np.eye(128)]
    gm_blocks, gf_blocks = [], []
    for j in range(RNK):
        Tn = np.zeros((L, L))
        Tf = np.zeros((L, L))
        for s_ in range(L):
            Tn[s_, s_:] = G[j, : L - s_]       # lhsT[s, t] = G_j(t - s)
            Tf[s_, :] = G[j, L - s_ : 2 * L - s_]  # lhsT[s, t] = G_j(t + L - s)
        gm_blocks.append(Tn)
        gf_blocks.append(Tf)
    blocks += gm_blocks + gf_blocks
    for j in range(RNK):
        blocks.append(np.broadcast_to(np.tile(w[:, j], 2)[None, :], (128, 2 * C)))
    cmat = np.concatenate(blocks, axis=1).astype(bf)  # [128, CW]
    consts = dict(cmat=cmat)
    return a, q, consts


def _beta_term(ln_beta, expansion, reduction, a, q):
    if not np.any(ln_beta):
        return None
    n_idx = np.arange(N, dtype=np.float64)
    Cn = a[:, None] * (1.0 - q[:, None] ** (n_idx[None, :] + 1.0)) / (1.0 - q[:, None])
    w = (
        expansion.astype(np.float64)
        * reduction.astype(np.float64)
        * ln_beta.astype(np.float64)[None, :]
    )
    return np.einsum("hc,hn->cn", w, Cn).astype(np.float32)


def _make_in_maps(x, consts):
    import ml_dtypes

    bf = ml_dtypes.bfloat16
    xt = np.ascontiguousarray(np.swapaxes(x, 1, 2)).astype(bf)  # [B, N, C]
    in_maps = []
    for core in range(N_CORES):
        b, half = divmod(core, 2)
        xs = np.zeros((NW, C), bf)
        s = half * NHALF - L
        if s < 0:
            xs[L:] = xt[b, :NHALF]
        else:
            xs[:] = xt[b, s : s + NW]
        # chunk-major [q, k, c] so device DMA slabs are contiguous/partition
        xs_km = np.ascontiguousarray(
            xs.reshape(NCH, 128, C).transpose(1, 0, 2)
        )
        in_maps.append(dict(consts, xs_t=xs_km))
    return in_maps


def kernel(x, ln_gamma, ln_beta, expansion, reduction, alphas, dampen_factors,
           trace=False):
    _install_ntff_shim()
    from concourse.bass_utils import run_bass_kernel_spmd
    from concourse.bass_interp import get_hw_module

    x = np.asarray(x, np.float32)
    a, q, consts = _host_params(
        np.asarray(ln_gamma), np.asarray(ln_beta), np.asarray(expansion),
        np.asarray(reduction), np.asarray(alphas), np.asarray(dampen_factors),
    )
    nc = build_program()
    _split_multiwait(nc)
    nc.m = get_hw_module(nc.m)

    in_maps = _make_in_maps(x, consts)
    res = run_bass_kernel_spmd(
        nc, in_maps, core_ids=list(range(N_CORES)), trace=trace
    )

    out = np.empty((B, C, N), np.float32)
    for core in range(N_CORES):
        b, half = divmod(core, 2)
        # [q, k, c] chunk-major -> [n, c] -> transpose to [c, n]
        ot = (res.results[core]["out_t"].astype(np.float32)
              .transpose(1, 0, 2).reshape(NHALF, C))
        out[b, :, half * NHALF : (half + 1) * NHALF] = ot.T
    bt = _beta_term(
        np.asarray(ln_beta), np.asarray(expansion), np.asarray(reduction), a, q
    )
    if bt is not None:
        out += bt[None]
    if trace:
        kernel.last_results = res
    return out
